# revision 6
# baseline (speedup 1.0000x reference)
"""Trainium2 Bass kernel for 2D MHSA with relative position logits (v2).

Per batch element b (8 total, one NeuronCore each — pure data parallel):
    qkv = w_qkv @ featuremap[b]
    per head n (8 heads, d=128):
      logits = (q*s) @ k^T + relpos(q*s)     # [1024, 1024]
      out[n] = softmax(logits) @ v           # [1024, 128]

v2 design (vs v1 baseline at ~307 us):
  - w_qkv transposed + bf16-converted on HOST; no on-device transposes.
    Constant tensors (wT, onehot, relT, identity) are DMA'd once, outside
    the BENCH_LOOP For_i loop.
  - All scale factors deferred to the exp: exp(S * raw_logit) via the Act
    engine's scale argument; q/k/rel all stored raw (fp8-friendly ranges).
  - QK^T and the rel-pos fold are merged into ONE fp8 DoubleRow matmul
    with K_eff=256: subtile 0 carries the full 128-dim q/k contraction,
    subtile 1 carries [zeros(64); onehot(64)] x [garbage(64); G(64)].
    PE cost: 0.5 cycles/output-col = 4x cheaper than bf16 QK + fold.
  - G gather matrices (rank-64 rel logits) built with 64 shifted-slice
    fp8 DoubleRow matmuls against host-prepared relT, folded into the
    Q8 moving tile's subtile-1 rows.
  - Softmax denominator: AV is computed flipped (out O[x, d], lhsT = E
    column blocks) against a V tile augmented with a ones-column, so
    Z[x] lands in PSUM column 128 of the same accumulation. No separate
    ones-matmul. O is normalized per-partition (x) then transposed back
    to [d, x] with PE transposes.
  - Output DRAM tensor is bf16; host converts to f32 (tolerance 2e-2).
  - Output DMAs issue from the (otherwise idle) gpsimd queue so they
    never head-of-line-block next-iteration input DMAs on the SP queue.
  - Optionally the For_i body holds TWO copies of the kernel with
    alternating buffer sets (UNROLL=2) so back-to-back iterations overlap
    across the loop edge despite the hardware loop reusing addresses.
"""

import os
import sys

for _p in ("/opt/trn_rl_repo", "/root/.axon_site/_ro/trn_rl_repo"):
    if os.path.isdir(_p) and _p not in sys.path:
        sys.path.append(_p)

import numpy as np

import concourse.bass as bass
import concourse.tile as tile
from concourse import bacc, mybir

F32 = mybir.dt.float32
BF16 = mybir.dt.bfloat16
F8 = mybir.dt.float8e4

B = 8          # batch == number of cores
NH = 8         # heads
D = 128        # head dim
H = 32
W = 32
HW = H * W     # 1024 positions
C = 512        # channels
SCALE = D ** -0.5

DR = mybir.MatmulPerfMode.DoubleRow
EXP = mybir.ActivationFunctionType.Exp


def build_nc(num_devices: int = B):
    nc = bacc.Bacc("TRN2", target_bir_lowering=False, debug=False,
                   num_devices=num_devices)

    f_d = nc.dram_tensor("f", [C, HW], BF16, kind="ExternalInput")
    w_d = nc.dram_tensor("wT", [C, 3 * NH * D], BF16, kind="ExternalInput")
    relw_d = nc.dram_tensor("relw", [128, 128], F8, kind="ExternalInput")
    relh_d = nc.dram_tensor("relh", [128, 128], F8, kind="ExternalInput")
    oh_d = nc.dram_tensor("oh", [128, NH * HW], F8, kind="ExternalInput")
    ident_d = nc.dram_tensor("ident", [128, 128], BF16, kind="ExternalInput")
    out_d = nc.dram_tensor("out", [NH * D, HW], BF16, kind="ExternalOutput")

    bench_loop = int(os.environ.get("BENCH_LOOP", "0"))
    unroll = 2 if bench_loop > 1 else 1
    from contextlib import ExitStack
    with tile.TileContext(nc) as tc:
        with ExitStack() as pools:
            st = _mk_state(nc, tc, pools, unroll)
            _load_consts(nc, st, w_d, relw_d, relh_d, oh_d, ident_d)
            if bench_loop > 1:
                assert bench_loop % unroll == 0
                with tc.For_i(0, bench_loop // unroll, 1):
                    for u in range(unroll):
                        _body(nc, st, u, f_d, out_d)
            else:
                _body(nc, st, 0, f_d, out_d)
    nc.compile()
    return nc


def _mk_state(nc, tc, pools, unroll):
    """Allocate all SBUF/PSUM pools. Per-iteration tiles (Q8/K8/v2/f) are
    allocated `unroll` times so unrolled bodies alternate buffers."""
    st = {}
    ctx = pools.enter_context

    big = ctx(tc.tile_pool(name="big", bufs=1))
    cst = ctx(tc.tile_pool(name="cst", bufs=1))

    for u in range(unroll):
        # fp8 DoubleRow operand tiles: [128 partitions, 2 subtiles, 8192]
        #   subtile 0: q/k, full d=128 on partitions
        #   subtile 1: rows 0-63 zero (K8) / garbage*0 (Q8), rows 64-127
        #              onehot (K8) / G gather values (Q8)
        st[f"Q8_{u}"] = big.tile([128, 2 * NH * HW], F8, tag=f"Q8_{u}",
                                 name=f"Q8_{u}")
        st[f"K8_{u}"] = big.tile([128, 2 * NH * HW], F8, tag=f"K8_{u}",
                                 name=f"K8_{u}")
        # V with ones column: [128 y, (j, n, 129)] bf16; col 128 == 1.0
        st[f"v2_{u}"] = big.tile([128, 8 * NH * 129], BF16, tag=f"v2_{u}",
                                 name=f"v2_{u}")
        st[f"f_{u}"] = [
            big.tile([128, HW], BF16, tag=f"f{i}_{u}", name=f"f{i}_{u}")
            for i in range(4)]

    st["ident"] = cst.tile([128, 128], BF16, tag="ident", name="ident")
    st["ones"] = cst.tile([128, 128], BF16, tag="ones", name="ones")
    st["relw"] = cst.tile([128, 128], F8, tag="relw", name="relw")
    st["relh"] = cst.tile([128, 128], F8, tag="relh", name="relh")
    st["wT"] = [cst.tile([128, 3 * NH * D], BF16, tag=f"wT{i}",
                         name=f"wT{i}") for i in range(4)]

    st["ep"] = ctx(tc.tile_pool(name="ep", bufs=14))
    st["rzp"] = ctx(tc.tile_pool(name="rzp", bufs=6))
    st["osp"] = ctx(tc.tile_pool(name="osp", bufs=6))
    st["obp"] = ctx(tc.tile_pool(name="obp", bufs=3))
    # PSUM: ps_l 2 x [128,1024]f32 (4 banks) for logits; psP 4 x
    # [128,512]f32 (4 banks) shared by projection halves, G outputs, AV
    # accumulation chains, and (via bf16 bitcast) transpose staging.
    st["ps_l"] = ctx(tc.tile_pool(name="ps_l", bufs=2,
                                  space=bass.MemorySpace.PSUM))
    st["psP"] = ctx(tc.tile_pool(name="psP", bufs=4,
                                 space=bass.MemorySpace.PSUM))
    st["unroll"] = unroll
    return st


def _load_consts(nc, st, w_d, relw_d, relh_d, oh_d, ident_d):
    for i in range(4):
        nc.sync.dma_start(st["wT"][i][:], w_d[i * 128:(i + 1) * 128, :])
    nc.sync.dma_start(st["relw"][:], relw_d[:])
    nc.sync.dma_start(st["relh"][:], relh_d[:])
    nc.sync.dma_start(st["ident"][:], ident_d[:])
    nc.gpsimd.memset(st["ones"][:], 1.0)
    for u in range(st["unroll"]):
        K8r = st[f"K8_{u}"].rearrange("p (i x) -> p i x", i=2)
        Q8r = st[f"Q8_{u}"].rearrange("p (i x) -> p i x", i=2)
        v2v = st[f"v2_{u}"].rearrange("p (j n c) -> p j n c", j=8, n=NH)
        # onehot constant -> K8 subtile 1 (rows 0-63 zero in the const)
        nc.sync.dma_start(K8r[:, 1, :], oh_d[:])
        # Q8 subtile 1 must be finite before the G matmuls read it (its
        # product is killed by K8's zeros, but NaN*0=NaN).
        nc.gpsimd.memset(Q8r[:, 1, :], 0.0)
        # ones column of the augmented V
        nc.gpsimd.memset(v2v[:, :, :, 128], 1.0)


def _body(nc, st, u, f_d, out_d):
    ident = st["ident"]
    ones = st["ones"]
    wT = st["wT"]
    f_sb = st[f"f_{u}"]
    Q8, K8, v2 = st[f"Q8_{u}"], st[f"K8_{u}"], st[f"v2_{u}"]
    ps_l, psP = st["ps_l"], st["psP"]
    ep, rzp, osp, obp = st["ep"], st["rzp"], st["osp"], st["obp"]

    Q8i = Q8.rearrange("p (i n x) -> p i n x", i=2, n=NH)
    Q8g = Q8.rearrange("p (i n h w) -> p i n h w", i=2, n=NH, h=H)
    K8i = K8.rearrange("p (i n x) -> p i n x", i=2, n=NH)
    v2v = v2.rearrange("p (j n c) -> p j n c", j=8, n=NH)
    rwv = st["relw"].rearrange("p (i j) -> p i j", i=2)
    rhv = st["relh"].rearrange("p (i j) -> p i j", i=2)

    for i in range(4):
        nc.sync.dma_start(f_sb[i][:], f_d[i * 128:(i + 1) * 128, :])

    def qk_proj(ob):
        n = ob % 8
        dst = Q8i if ob < 8 else K8i
        for ch in range(2):
            ps = psP.tile([128, 512], F32, tag="pP",
                          name=f"pj{u}_{ob}_{ch}")
            for cb in range(4):
                nc.tensor.matmul(
                    ps[:],
                    wT[cb][:, ob * 128:(ob + 1) * 128],
                    f_sb[cb][:, ch * 512:(ch + 1) * 512],
                    start=(cb == 0), stop=(cb == 3))
            nc.vector.tensor_copy(dst[:, 0, n, ch * 512:(ch + 1) * 512],
                                  ps[:])

    def v_proj(yb):
        for oc in range(2):
            ps = psP.tile([128, 512], F32, tag="pP", name=f"pv{u}_{yb}_{oc}")
            for cb in range(4):
                nc.tensor.matmul(
                    ps[:],
                    f_sb[cb][:, yb * 128:(yb + 1) * 128],
                    wT[cb][:, 2048 + oc * 512:2048 + (oc + 1) * 512],
                    start=(cb == 0), stop=(cb == 3))
            psv = ps.rearrange("p (n d) -> p n d", n=4)
            nc.vector.tensor_copy(v2v[:, yb, oc * 4:(oc + 1) * 4, 0:128],
                                  psv[:])

    E = {}

    def qk_head(n, j):
        ps = ps_l.tile([128, 1024], F32, tag="l", name=f"l{u}_{n}_{j}")
        for ch in range(2):
            nc.tensor.matmul(
                ps[:, ch * 512:(ch + 1) * 512],
                K8i[:, :, n, j * 128:(j + 1) * 128],
                Q8i[:, :, n, ch * 512:(ch + 1) * 512],
                start=True, stop=True, perf_mode=DR)
        e = ep.tile([128, 1024], BF16, tag="e", name=f"e{u}_{n}_{j}")
        nc.scalar.activation(e[:], ps[:], EXP, scale=SCALE)
        E[(n, j)] = e

    # ---- q projections, then G, then pipelined attention ---------------
    for ob in range(8):
        qk_proj(ob)

    # G gather matrices into Q8 subtile 1:
    # Gw[b, x] = Lw[x, b - w(x) + 31] -> rows 64-95; Gh -> rows 96-127.
    # Two sub-phases (all Gw, then all Gh): a Gh matmul's read of
    # Q8[:, 1, (n, hh, :)] overlaps every Gw eviction, so Gh waits for
    # the Gw sub-phase; within a sub-phase, column classes (w(x) == ww)
    # are disjoint. Evictions alternate DVE/Act to halve the phase wall.
    for ww in range(W):
        ps = psP.tile([128, 512], F32, tag="pP", name=f"gw{u}_{ww}")
        nc.tensor.matmul(ps[0:32, 0:256], rwv[:, :, 31 - ww:63 - ww],
                         Q8g[:, :, :, :, ww], start=True, stop=True,
                         perf_mode=DR)
        if ww % 2 == 0:
            nc.vector.tensor_copy(Q8g[64:96, 1, :, :, ww], ps[0:32, 0:256])
        else:
            nc.scalar.copy(Q8g[64:96, 1, :, :, ww], ps[0:32, 0:256])
    for hh in range(H):
        ps = psP.tile([128, 512], F32, tag="pP", name=f"gh{u}_{hh}")
        nc.tensor.matmul(ps[0:32, 0:256], rhv[:, :, 31 - hh:63 - hh],
                         Q8g[:, :, :, hh, :], start=True, stop=True,
                         perf_mode=DR)
        if hh % 2 == 0:
            nc.vector.tensor_copy(Q8g[96:128, 1, :, hh, :], ps[0:32, 0:256])
        else:
            nc.scalar.copy(Q8g[96:128, 1, :, hh, :], ps[0:32, 0:256])

    # first k head + its logits
    qk_proj(8)
    for j in range(8):
        qk_head(0, j)

    for n in range(NH):
        # lookahead: next head's k projection + first 4 logit tiles keeps
        # the Act exp chain fed while this head's AV runs
        if n + 1 < NH:
            qk_proj(8 + n + 1)
            for j in range(4):
                qk_head(n + 1, j)
        if n == 0:
            for yb in range(8):
                v_proj(yb)

        # wide AV (out O^T[d, x], lhsT = V(j, n) shared across both ch
        # halves -> half the weight loads) + ones-matmul Z (constant lhsT
        # across the whole chain). The flip-AV variant modeled faster but
        # measured slower on HW (unmodeled per-matmul weight-load cost).
        pso = [psP.tile([128, 512], F32, tag="pP", name=f"o{u}_{n}_{ch}")
               for ch in range(2)]
        for j in range(8):
            for ch in range(2):
                nc.tensor.matmul(
                    pso[ch][:],
                    v2v[:, j, n, 0:128],
                    E[(n, j)][:, ch * 512:(ch + 1) * 512],
                    start=(j == 0), stop=(j == 7))
        psz = [psP.tile([128, 512], F32, tag="pP", name=f"z{u}_{n}_{ch}")
               for ch in range(2)]
        for j in range(8):
            for ch in range(2):
                nc.tensor.matmul(
                    psz[ch][:],
                    ones[:],
                    E[(n, j)][:, ch * 512:(ch + 1) * 512],
                    start=(j == 0), stop=(j == 7))
        for ch in range(2):
            rz = rzp.tile([128, 512], F32, tag="rz", name=f"rz{u}_{n}_{ch}")
            nc.vector.reciprocal(rz[:], psz[ch][:])
            ob = obp.tile([128, 512], BF16, tag="ob", name=f"ob{u}_{n}_{ch}")
            nc.vector.tensor_mul(ob[:], pso[ch][:], rz[:])
            # out-DMAs go via the idle gpsimd queue: SP's in-order DMA
            # queue must stay clear for next-iteration input DMAs
            nc.gpsimd.dma_start(
                out_d[n * 128:(n + 1) * 128, ch * 512:(ch + 1) * 512],
                ob[:])

        if n + 1 < NH:
            for j in range(4, 8):
                qk_head(n + 1, j)


def _consts():
    import ml_dtypes
    F8NP = ml_dtypes.float8_e4m3
    BFNP = ml_dtypes.bfloat16

    def rel_split(rel):
        # [128, 2, 64] fp8: subtile 0 = rel^T (full d=128), subtile 1 = 0
        pad = np.zeros((64, 128), np.float32)
        pad[:63] = rel
        t = np.zeros((128, 2, 64), np.float32)
        t[:, 0, :] = pad.T            # [d, j]
        return np.ascontiguousarray(t.reshape(128, 128)).astype(F8NP)

    x = np.arange(HW)
    hh, ww = np.divmod(x, W)
    oh = np.zeros((128, NH, HW), np.float32)
    for b in range(32):
        oh[64 + b, :, :] = (ww == b)[None, :]
        oh[96 + b, :, :] = (hh == b)[None, :]
    oh = oh.reshape(128, NH * HW).astype(F8NP)

    ident = np.eye(128, dtype=np.float32).astype(BFNP)
    return rel_split, oh, ident


def make_in_maps(featuremap, w_qkv, rel_height, rel_width):
    import ml_dtypes
    BFNP = ml_dtypes.bfloat16

    rel_split, oh, ident = _consts()
    wT = np.ascontiguousarray(np.asarray(w_qkv, np.float32).T).astype(BFNP)
    relw = rel_split(np.asarray(rel_width, np.float32))
    relh = rel_split(np.asarray(rel_height, np.float32))
    fm = np.asarray(featuremap, np.float32)
    maps = []
    for b in range(B):
        maps.append({
            "f": np.ascontiguousarray(fm[b].reshape(C, HW)).astype(BFNP),
            "wT": wT, "relw": relw, "relh": relh,
            "oh": oh, "ident": ident,
        })
    return maps


_NC_CACHE = {}


def get_nc():
    if "nc" not in _NC_CACHE:
        _NC_CACHE["nc"] = build_nc()
    return _NC_CACHE["nc"]


def kernel(featuremap, w_qkv, rel_height, rel_width):
    from concourse.bass_utils import run_bass_kernel_spmd

    nc = get_nc()
    in_maps = make_in_maps(featuremap, w_qkv, rel_height, rel_width)
    res = run_bass_kernel_spmd(nc, in_maps, list(range(B)))
    out = np.stack([np.asarray(res.results[b]["out"], dtype=np.float32)
                    for b in range(B)])
    return out.reshape(B, NH * D, H, W)


if __name__ == "__main__":
    nc = build_nc()
    print("built ok:", len(nc.m.functions[0].blocks), "blocks")


# revision 7
# speedup vs baseline: 1.4452x; 1.4452x over previous
"""Trainium2 Bass kernel for 2D MHSA with relative position logits (v2).

Per batch element b (8 total, one NeuronCore each — pure data parallel):
    qkv = w_qkv @ featuremap[b]
    per head n (8 heads, d=128):
      logits = (q*s) @ k^T + relpos(q*s)     # [1024, 1024]
      out[n] = softmax(logits) @ v           # [1024, 128]

v2 design (vs v1 baseline at ~307 us):
  - w_qkv transposed + bf16-converted on HOST; no on-device transposes.
    Constant tensors (wT, onehot, relT, identity) are DMA'd once, outside
    the BENCH_LOOP For_i loop.
  - All scale factors deferred to the exp: exp(S * raw_logit) via the Act
    engine's scale argument; q/k/rel all stored raw (fp8-friendly ranges).
  - QK^T and the rel-pos fold are merged into ONE fp8 DoubleRow matmul
    with K_eff=256: subtile 0 carries the full 128-dim q/k contraction,
    subtile 1 carries [zeros(64); onehot(64)] x [garbage(64); G(64)].
    PE cost: 0.5 cycles/output-col = 4x cheaper than bf16 QK + fold.
  - G gather matrices (rank-64 rel logits) built with 64 shifted-slice
    fp8 DoubleRow matmuls against host-prepared relT, folded into the
    Q8 moving tile's subtile-1 rows.
  - Softmax denominator: AV is computed flipped (out O[x, d], lhsT = E
    column blocks) against a V tile augmented with a ones-column, so
    Z[x] lands in PSUM column 128 of the same accumulation. No separate
    ones-matmul. O is normalized per-partition (x) then transposed back
    to [d, x] with PE transposes.
  - Output DRAM tensor is bf16; host converts to f32 (tolerance 2e-2).
  - Output DMAs issue from the (otherwise idle) gpsimd queue so they
    never head-of-line-block next-iteration input DMAs on the SP queue.
  - Optionally the For_i body holds TWO copies of the kernel with
    alternating buffer sets (UNROLL=2) so back-to-back iterations overlap
    across the loop edge despite the hardware loop reusing addresses.
"""

import os
import sys

for _p in ("/opt/trn_rl_repo", "/root/.axon_site/_ro/trn_rl_repo"):
    if os.path.isdir(_p) and _p not in sys.path:
        sys.path.append(_p)

import numpy as np

import concourse.bass as bass
import concourse.tile as tile
from concourse import bacc, mybir

F32 = mybir.dt.float32
BF16 = mybir.dt.bfloat16
F8 = mybir.dt.float8e4

B = 8          # batch == number of cores
NH = 8         # heads
D = 128        # head dim
H = 32
W = 32
HW = H * W     # 1024 positions
C = 512        # channels
SCALE = D ** -0.5

DR = mybir.MatmulPerfMode.DoubleRow
EXP = mybir.ActivationFunctionType.Exp


def build_nc(num_devices: int = B):
    nc = bacc.Bacc("TRN2", target_bir_lowering=False, debug=False,
                   num_devices=num_devices)

    f_d = nc.dram_tensor("f", [C, HW], BF16, kind="ExternalInput")
    w_d = nc.dram_tensor("wT", [C, 3 * NH * D], BF16, kind="ExternalInput")
    relw_d = nc.dram_tensor("relw", [128, 128], F8, kind="ExternalInput")
    relh_d = nc.dram_tensor("relh", [128, 128], F8, kind="ExternalInput")
    oh_d = nc.dram_tensor("oh", [128, NH * HW], F8, kind="ExternalInput")
    ident_d = nc.dram_tensor("ident", [128, 128], BF16, kind="ExternalInput")
    out_d = nc.dram_tensor("out", [NH * D, HW], BF16, kind="ExternalOutput")

    bench_loop = int(os.environ.get("BENCH_LOOP", "0"))
    unroll = 2 if bench_loop > 1 else 1
    from contextlib import ExitStack
    with tile.TileContext(nc) as tc:
        with ExitStack() as pools:
            st = _mk_state(nc, tc, pools, unroll)
            _load_consts(nc, st, w_d, relw_d, relh_d, oh_d, ident_d)
            if bench_loop > 1:
                assert bench_loop % unroll == 0
                with tc.For_i(0, bench_loop // unroll, 1):
                    for u in range(unroll):
                        _body(nc, st, u, f_d, out_d)
            else:
                _body(nc, st, 0, f_d, out_d)
    nc.compile()
    return nc


def _mk_state(nc, tc, pools, unroll):
    """Allocate all SBUF/PSUM pools. Per-iteration tiles (Q8/K8/v2/f) are
    allocated `unroll` times so unrolled bodies alternate buffers."""
    st = {}
    ctx = pools.enter_context

    big = ctx(tc.tile_pool(name="big", bufs=1))
    cst = ctx(tc.tile_pool(name="cst", bufs=1))

    for u in range(unroll):
        # fp8 DoubleRow operand tiles: [128 partitions, 2 subtiles, 8192]
        #   subtile 0: q/k, full d=128 on partitions
        #   subtile 1: rows 0-63 zero (K8) / garbage*0 (Q8), rows 64-127
        #              onehot (K8) / G gather values (Q8)
        st[f"Q8_{u}"] = big.tile([128, 2 * NH * HW], F8, tag=f"Q8_{u}",
                                 name=f"Q8_{u}")
        st[f"K8_{u}"] = big.tile([128, 2 * NH * HW], F8, tag=f"K8_{u}",
                                 name=f"K8_{u}")
        # V with ones column: [128 y, (j, n, 129)] bf16; col 128 == 1.0
        st[f"v2_{u}"] = big.tile([128, 8 * NH * 129], BF16, tag=f"v2_{u}",
                                 name=f"v2_{u}")
        st[f"f_{u}"] = [
            big.tile([128, HW], BF16, tag=f"f{i}_{u}", name=f"f{i}_{u}")
            for i in range(4)]

    st["ident"] = cst.tile([128, 128], BF16, tag="ident", name="ident")
    st["relw"] = cst.tile([128, 128], F8, tag="relw", name="relw")
    st["relh"] = cst.tile([128, 128], F8, tag="relh", name="relh")
    st["wT"] = [cst.tile([128, 3 * NH * D], BF16, tag=f"wT{i}",
                         name=f"wT{i}") for i in range(4)]

    st["ep"] = ctx(tc.tile_pool(name="ep", bufs=14))
    st["rzp"] = ctx(tc.tile_pool(name="rzp", bufs=6))
    st["osp"] = ctx(tc.tile_pool(name="osp", bufs=6))
    st["obp"] = ctx(tc.tile_pool(name="obp", bufs=3))
    # PSUM: ps_l 2 x [128,1024]f32 (4 banks) for logits; psP 4 x
    # [128,512]f32 (4 banks) shared by projection halves, G outputs, AV
    # accumulation chains, and (via bf16 bitcast) transpose staging.
    st["ps_l"] = ctx(tc.tile_pool(name="ps_l", bufs=2,
                                  space=bass.MemorySpace.PSUM))
    st["psP"] = ctx(tc.tile_pool(name="psP", bufs=4,
                                 space=bass.MemorySpace.PSUM))
    st["unroll"] = unroll
    return st


def _load_consts(nc, st, w_d, relw_d, relh_d, oh_d, ident_d):
    for i in range(4):
        nc.sync.dma_start(st["wT"][i][:], w_d[i * 128:(i + 1) * 128, :])
    nc.sync.dma_start(st["relw"][:], relw_d[:])
    nc.sync.dma_start(st["relh"][:], relh_d[:])
    nc.sync.dma_start(st["ident"][:], ident_d[:])
    for u in range(st["unroll"]):
        K8r = st[f"K8_{u}"].rearrange("p (i x) -> p i x", i=2)
        Q8r = st[f"Q8_{u}"].rearrange("p (i x) -> p i x", i=2)
        v2v = st[f"v2_{u}"].rearrange("p (j n c) -> p j n c", j=8, n=NH)
        # onehot constant -> K8 subtile 1 (rows 0-63 zero in the const)
        nc.sync.dma_start(K8r[:, 1, :], oh_d[:])
        # Q8 subtile 1 must be finite before the G matmuls read it (its
        # product is killed by K8's zeros, but NaN*0=NaN).
        nc.gpsimd.memset(Q8r[:, 1, :], 0.0)
        # ones column of the augmented V
        nc.gpsimd.memset(v2v[:, :, :, 128], 1.0)


def _body(nc, st, u, f_d, out_d):
    ident = st["ident"]
    wT = st["wT"]
    f_sb = st[f"f_{u}"]
    Q8, K8, v2 = st[f"Q8_{u}"], st[f"K8_{u}"], st[f"v2_{u}"]
    ps_l, psP = st["ps_l"], st["psP"]
    ep, rzp, osp, obp = st["ep"], st["rzp"], st["osp"], st["obp"]

    Q8i = Q8.rearrange("p (i n x) -> p i n x", i=2, n=NH)
    Q8g = Q8.rearrange("p (i n h w) -> p i n h w", i=2, n=NH, h=H)
    K8i = K8.rearrange("p (i n x) -> p i n x", i=2, n=NH)
    v2v = v2.rearrange("p (j n c) -> p j n c", j=8, n=NH)
    rwv = st["relw"].rearrange("p (i j) -> p i j", i=2)
    rhv = st["relh"].rearrange("p (i j) -> p i j", i=2)

    for i in range(4):
        nc.sync.dma_start(f_sb[i][:], f_d[i * 128:(i + 1) * 128, :])

    def qk_proj(ob):
        n = ob % 8
        dst = Q8i if ob < 8 else K8i
        for ch in range(2):
            ps = psP.tile([128, 512], F32, tag="pP",
                          name=f"pj{u}_{ob}_{ch}")
            for cb in range(4):
                nc.tensor.matmul(
                    ps[:],
                    wT[cb][:, ob * 128:(ob + 1) * 128],
                    f_sb[cb][:, ch * 512:(ch + 1) * 512],
                    start=(cb == 0), stop=(cb == 3))
            nc.vector.tensor_copy(dst[:, 0, n, ch * 512:(ch + 1) * 512],
                                  ps[:])

    def v_proj(yb):
        for oc in range(2):
            ps = psP.tile([128, 512], F32, tag="pP", name=f"pv{u}_{yb}_{oc}")
            for cb in range(4):
                nc.tensor.matmul(
                    ps[:],
                    f_sb[cb][:, yb * 128:(yb + 1) * 128],
                    wT[cb][:, 2048 + oc * 512:2048 + (oc + 1) * 512],
                    start=(cb == 0), stop=(cb == 3))
            psv = ps.rearrange("p (n d) -> p n d", n=4)
            nc.vector.tensor_copy(v2v[:, yb, oc * 4:(oc + 1) * 4, 0:128],
                                  psv[:])

    E = {}

    def qk_head(n, j):
        ps = ps_l.tile([128, 1024], F32, tag="l", name=f"l{u}_{n}_{j}")
        for ch in range(2):
            nc.tensor.matmul(
                ps[:, ch * 512:(ch + 1) * 512],
                K8i[:, :, n, j * 128:(j + 1) * 128],
                Q8i[:, :, n, ch * 512:(ch + 1) * 512],
                start=True, stop=True, perf_mode=DR)
        e = ep.tile([128, 1024], BF16, tag="e", name=f"e{u}_{n}_{j}")
        nc.scalar.activation(e[:], ps[:], EXP, scale=SCALE)
        E[(n, j)] = e

    # ---- q projections, then G, then pipelined attention ---------------
    for ob in range(8):
        qk_proj(ob)

    # G gather matrices into Q8 subtile 1:
    # Gw[b, x] = Lw[x, b - w(x) + 31] -> rows 64-95; Gh -> rows 96-127.
    # Two sub-phases (all Gw, then all Gh): a Gh matmul's read of
    # Q8[:, 1, (n, hh, :)] overlaps every Gw eviction, so Gh waits for
    # the Gw sub-phase; within a sub-phase, column classes (w(x) == ww)
    # are disjoint. Evictions alternate DVE/Act to halve the phase wall.
    for ww in range(W):
        ps = psP.tile([128, 512], F32, tag="pP", name=f"gw{u}_{ww}")
        nc.tensor.matmul(ps[0:32, 0:256], rwv[:, :, 31 - ww:63 - ww],
                         Q8g[:, :, :, :, ww], start=True, stop=True,
                         perf_mode=DR)
        if ww % 2 == 0:
            nc.vector.tensor_copy(Q8g[64:96, 1, :, :, ww], ps[0:32, 0:256])
        else:
            nc.scalar.copy(Q8g[64:96, 1, :, :, ww], ps[0:32, 0:256])
    for hh in range(H):
        ps = psP.tile([128, 512], F32, tag="pP", name=f"gh{u}_{hh}")
        nc.tensor.matmul(ps[0:32, 0:256], rhv[:, :, 31 - hh:63 - hh],
                         Q8g[:, :, :, hh, :], start=True, stop=True,
                         perf_mode=DR)
        if hh % 2 == 0:
            nc.vector.tensor_copy(Q8g[96:128, 1, :, hh, :], ps[0:32, 0:256])
        else:
            nc.scalar.copy(Q8g[96:128, 1, :, hh, :], ps[0:32, 0:256])

    # first k head + its logits
    qk_proj(8)
    for j in range(8):
        qk_head(0, j)

    for n in range(NH):
        # lookahead: next head's k projection + first 4 logit tiles keeps
        # the Act exp chain fed while this head's AV runs
        if n + 1 < NH:
            qk_proj(8 + n + 1)
            for j in range(4):
                qk_head(n + 1, j)
        if n == 0:
            for yb in range(8):
                v_proj(yb)

        for g4 in range(2):
            # transpose staging: bf16 view of a psP slot
            ptt = psP.tile([128, 512], F32, tag="pP", name=f"pt{u}_{n}_{g4}")
            pt = ptt.bitcast(BF16)[:, 0:512]
            for pair in range(2):
                pso = psP.tile([128, 512], F32, tag="pP",
                               name=f"o{u}_{n}_{g4}_{pair}")
                for sub in range(2):
                    xb = g4 * 4 + pair * 2 + sub
                    reg = pso[:, sub * 129:(sub + 1) * 129]
                    for j in range(8):
                        nc.tensor.matmul(
                            reg,
                            E[(n, j)][:, xb * 128:(xb + 1) * 128],
                            v2v[:, j, n, :],
                            start=(j == 0), stop=(j == 7))
                for sub in range(2):
                    xb = g4 * 4 + pair * 2 + sub
                    c0 = sub * 129
                    rz = rzp.tile([128, 1], F32, tag="rz",
                                  name=f"rz{u}_{n}_{xb}")
                    nc.vector.reciprocal(rz[:], pso[:, c0 + 128:c0 + 129])
                    osb = osp.tile([128, 128], BF16, tag="osb",
                                   name=f"osb{u}_{n}_{xb}")
                    nc.vector.tensor_scalar_mul(osb[:], pso[:, c0:c0 + 128],
                                                rz[:])
                    nc.tensor.transpose(
                        pt[:, (pair * 2 + sub) * 128:
                           (pair * 2 + sub + 1) * 128],
                        osb[:], ident[:])
            ob = obp.tile([128, 512], BF16, tag="ob", name=f"ob{u}_{n}_{g4}")
            nc.vector.tensor_copy(ob[:], pt[:])
            # out-DMAs go via the idle gpsimd queue: SP's in-order DMA
            # queue must stay clear for next-iteration input DMAs
            nc.gpsimd.dma_start(
                out_d[n * 128:(n + 1) * 128, g4 * 512:(g4 + 1) * 512],
                ob[:])

        if n + 1 < NH:
            for j in range(4, 8):
                qk_head(n + 1, j)


def _consts():
    import ml_dtypes
    F8NP = ml_dtypes.float8_e4m3
    BFNP = ml_dtypes.bfloat16

    def rel_split(rel):
        # [128, 2, 64] fp8: subtile 0 = rel^T (full d=128), subtile 1 = 0
        pad = np.zeros((64, 128), np.float32)
        pad[:63] = rel
        t = np.zeros((128, 2, 64), np.float32)
        t[:, 0, :] = pad.T            # [d, j]
        return np.ascontiguousarray(t.reshape(128, 128)).astype(F8NP)

    x = np.arange(HW)
    hh, ww = np.divmod(x, W)
    oh = np.zeros((128, NH, HW), np.float32)
    for b in range(32):
        oh[64 + b, :, :] = (ww == b)[None, :]
        oh[96 + b, :, :] = (hh == b)[None, :]
    oh = oh.reshape(128, NH * HW).astype(F8NP)

    ident = np.eye(128, dtype=np.float32).astype(BFNP)
    return rel_split, oh, ident


def make_in_maps(featuremap, w_qkv, rel_height, rel_width):
    import ml_dtypes
    BFNP = ml_dtypes.bfloat16

    rel_split, oh, ident = _consts()
    wT = np.ascontiguousarray(np.asarray(w_qkv, np.float32).T).astype(BFNP)
    relw = rel_split(np.asarray(rel_width, np.float32))
    relh = rel_split(np.asarray(rel_height, np.float32))
    fm = np.asarray(featuremap, np.float32)
    maps = []
    for b in range(B):
        maps.append({
            "f": np.ascontiguousarray(fm[b].reshape(C, HW)).astype(BFNP),
            "wT": wT, "relw": relw, "relh": relh,
            "oh": oh, "ident": ident,
        })
    return maps


_NC_CACHE = {}


def get_nc():
    if "nc" not in _NC_CACHE:
        _NC_CACHE["nc"] = build_nc()
    return _NC_CACHE["nc"]


def kernel(featuremap, w_qkv, rel_height, rel_width):
    from concourse.bass_utils import run_bass_kernel_spmd

    nc = get_nc()
    in_maps = make_in_maps(featuremap, w_qkv, rel_height, rel_width)
    res = run_bass_kernel_spmd(nc, in_maps, list(range(B)))
    out = np.stack([np.asarray(res.results[b]["out"], dtype=np.float32)
                    for b in range(B)])
    return out.reshape(B, NH * D, H, W)


if __name__ == "__main__":
    nc = build_nc()
    print("built ok:", len(nc.m.functions[0].blocks), "blocks")


# revision 8
# speedup vs baseline: 4.5044x; 3.1168x over previous
"""Trainium2 Bass kernel for 2D MHSA with relative position logits (v2).

Per batch element b (8 total, one NeuronCore each — pure data parallel):
    qkv = w_qkv @ featuremap[b]
    per head n (8 heads, d=128):
      logits = (q*s) @ k^T + relpos(q*s)     # [1024, 1024]
      out[n] = softmax(logits) @ v           # [1024, 128]

v2 design (vs v1 baseline at ~307 us):
  - w_qkv transposed + bf16-converted on HOST; no on-device transposes.
    Constant tensors (wT, onehot, relT, identity) are DMA'd once, outside
    the BENCH_LOOP For_i loop.
  - All scale factors deferred to the exp: exp(S * raw_logit) via the Act
    engine's scale argument; q/k/rel all stored raw (fp8-friendly ranges).
  - QK^T and the rel-pos fold are merged into ONE fp8 DoubleRow matmul
    with K_eff=256: subtile 0 carries the full 128-dim q/k contraction,
    subtile 1 carries [zeros(64); onehot(64)] x [garbage(64); G(64)].
    PE cost: 0.5 cycles/output-col = 4x cheaper than bf16 QK + fold.
  - G gather matrices (rank-64 rel logits) built with 64 shifted-slice
    fp8 DoubleRow matmuls against host-prepared relT, folded into the
    Q8 moving tile's subtile-1 rows.
  - Softmax denominator: AV is computed flipped (out O[x, d], lhsT = E
    column blocks) against a V tile augmented with a ones-column, so
    Z[x] lands in PSUM column 128 of the same accumulation. No separate
    ones-matmul. O is normalized per-partition (x) then transposed back
    to [d, x] with PE transposes.
  - Output DRAM tensor is bf16; host converts to f32 (tolerance 2e-2).
  - Output DMAs issue from the (otherwise idle) gpsimd queue so they
    never head-of-line-block next-iteration input DMAs on the SP queue.
  - Optionally the For_i body holds TWO copies of the kernel with
    alternating buffer sets (UNROLL=2) so back-to-back iterations overlap
    across the loop edge despite the hardware loop reusing addresses.
"""

import os
import sys

for _p in ("/opt/trn_rl_repo", "/root/.axon_site/_ro/trn_rl_repo"):
    if os.path.isdir(_p) and _p not in sys.path:
        sys.path.append(_p)

import numpy as np

import concourse.bass as bass
import concourse.tile as tile
from concourse import bacc, mybir

F32 = mybir.dt.float32
BF16 = mybir.dt.bfloat16
F8 = mybir.dt.float8e4

B = 8          # batch == number of cores
NH = 8         # heads
D = 128        # head dim
H = 32
W = 32
HW = H * W     # 1024 positions
C = 512        # channels
SCALE = D ** -0.5

DR = mybir.MatmulPerfMode.DoubleRow
EXP = mybir.ActivationFunctionType.Exp


def build_nc(num_devices: int = B):
    nc = bacc.Bacc("TRN2", target_bir_lowering=False, debug=False,
                   num_devices=num_devices)

    f_d = nc.dram_tensor("f", [C, HW], BF16, kind="ExternalInput")
    w_d = nc.dram_tensor("wT", [C, 3 * NH * D], BF16, kind="ExternalInput")
    relw_d = nc.dram_tensor("relw", [128, 128], F8, kind="ExternalInput")
    relh_d = nc.dram_tensor("relh", [128, 128], F8, kind="ExternalInput")
    oh_d = nc.dram_tensor("oh", [128, NH * HW], F8, kind="ExternalInput")
    ident_d = nc.dram_tensor("ident", [128, 128], BF16, kind="ExternalInput")
    out_d = nc.dram_tensor("out", [NH * D, HW], BF16, kind="ExternalOutput")

    bench_loop = int(os.environ.get("BENCH_LOOP", "0"))
    unroll = 2 if bench_loop > 1 else 1
    from contextlib import ExitStack
    with tile.TileContext(nc) as tc:
        with ExitStack() as pools:
            st = _mk_state(nc, tc, pools, unroll)
            _load_consts(nc, st, w_d, relw_d, relh_d, oh_d, ident_d)
            if bench_loop > 1:
                assert bench_loop % unroll == 0
                with tc.For_i(0, bench_loop // unroll, 1):
                    for u in range(unroll):
                        _body(nc, st, u, f_d, out_d)
            else:
                _body(nc, st, 0, f_d, out_d)
    nc.compile()
    return nc


def _mk_state(nc, tc, pools, unroll):
    """Allocate all SBUF/PSUM pools. Per-iteration tiles (Q8/K8/v2/f) are
    allocated `unroll` times so unrolled bodies alternate buffers."""
    st = {}
    ctx = pools.enter_context

    big = ctx(tc.tile_pool(name="big", bufs=1))
    cst = ctx(tc.tile_pool(name="cst", bufs=1))

    for u in range(unroll):
        # fp8 DoubleRow operand tiles: [128 partitions, 2 subtiles, 8192]
        #   subtile 0: q/k, full d=128 on partitions
        #   subtile 1: rows 0-63 zero (K8) / garbage*0 (Q8), rows 64-127
        #              onehot (K8) / G gather values (Q8)
        st[f"Q8_{u}"] = big.tile([128, 2 * NH * HW], F8, tag=f"Q8_{u}",
                                 name=f"Q8_{u}")
        st[f"K8_{u}"] = big.tile([128, 2 * NH * HW], F8, tag=f"K8_{u}",
                                 name=f"K8_{u}")
        # V with ones column: [128 y, (j, n, 129)] bf16; col 128 == 1.0
        st[f"v2_{u}"] = big.tile([128, 8 * NH * 129], BF16, tag=f"v2_{u}",
                                 name=f"v2_{u}")
        st[f"f_{u}"] = [
            big.tile([128, HW], BF16, tag=f"f{i}_{u}", name=f"f{i}_{u}")
            for i in range(4)]

    st["ident"] = cst.tile([128, 128], BF16, tag="ident", name="ident")
    st["relw"] = cst.tile([128, 128], F8, tag="relw", name="relw")
    st["relh"] = cst.tile([128, 128], F8, tag="relh", name="relh")
    st["wT"] = [cst.tile([128, 3 * NH * D], BF16, tag=f"wT{i}",
                         name=f"wT{i}") for i in range(4)]

    st["ep"] = ctx(tc.tile_pool(name="ep", bufs=14))
    st["rzp"] = ctx(tc.tile_pool(name="rzp", bufs=6))
    st["osp"] = ctx(tc.tile_pool(name="osp", bufs=6))
    st["obp"] = ctx(tc.tile_pool(name="obp", bufs=3))
    # PSUM: ps_l 2 x [128,1024]f32 (4 banks) for logits; psP 4 x
    # [128,512]f32 (4 banks) shared by projection halves, G outputs, AV
    # accumulation chains, and (via bf16 bitcast) transpose staging.
    st["ps_l"] = ctx(tc.tile_pool(name="ps_l", bufs=2,
                                  space=bass.MemorySpace.PSUM))
    st["psP"] = ctx(tc.tile_pool(name="psP", bufs=4,
                                 space=bass.MemorySpace.PSUM))
    st["unroll"] = unroll
    return st


def _load_consts(nc, st, w_d, relw_d, relh_d, oh_d, ident_d):
    for i in range(4):
        nc.sync.dma_start(st["wT"][i][:], w_d[i * 128:(i + 1) * 128, :])
    nc.sync.dma_start(st["relw"][:], relw_d[:])
    nc.sync.dma_start(st["relh"][:], relh_d[:])
    nc.sync.dma_start(st["ident"][:], ident_d[:])
    for u in range(st["unroll"]):
        K8r = st[f"K8_{u}"].rearrange("p (i x) -> p i x", i=2)
        Q8r = st[f"Q8_{u}"].rearrange("p (i x) -> p i x", i=2)
        v2v = st[f"v2_{u}"].rearrange("p (j n c) -> p j n c", j=8, n=NH)
        # onehot constant -> K8 subtile 1 (rows 0-63 zero in the const)
        nc.sync.dma_start(K8r[:, 1, :], oh_d[:])
        # Q8 subtile 1 must be finite before the G matmuls read it (its
        # product is killed by K8's zeros, but NaN*0=NaN).
        nc.gpsimd.memset(Q8r[:, 1, :], 0.0)
        # ones column of the augmented V
        nc.gpsimd.memset(v2v[:, :, :, 128], 1.0)


def _body(nc, st, u, f_d, out_d):
    ident = st["ident"]
    wT = st["wT"]
    f_sb = st[f"f_{u}"]
    Q8, K8, v2 = st[f"Q8_{u}"], st[f"K8_{u}"], st[f"v2_{u}"]
    ps_l, psP = st["ps_l"], st["psP"]
    ep, rzp, osp, obp = st["ep"], st["rzp"], st["osp"], st["obp"]

    Q8i = Q8.rearrange("p (i n x) -> p i n x", i=2, n=NH)
    Q8g = Q8.rearrange("p (i n h w) -> p i n h w", i=2, n=NH, h=H)
    K8i = K8.rearrange("p (i n x) -> p i n x", i=2, n=NH)
    v2v = v2.rearrange("p (j n c) -> p j n c", j=8, n=NH)
    rwv = st["relw"].rearrange("p (i j) -> p i j", i=2)
    rhv = st["relh"].rearrange("p (i j) -> p i j", i=2)

    for i in range(4):
        nc.sync.dma_start(f_sb[i][:], f_d[i * 128:(i + 1) * 128, :])

    def qk_proj(ob):
        n = ob % 8
        dst = Q8i if ob < 8 else K8i
        for ch in range(2):
            ps = psP.tile([128, 512], F32, tag="pP",
                          name=f"pj{u}_{ob}_{ch}")
            for cb in range(4):
                nc.tensor.matmul(
                    ps[:],
                    wT[cb][:, ob * 128:(ob + 1) * 128],
                    f_sb[cb][:, ch * 512:(ch + 1) * 512],
                    start=(cb == 0), stop=(cb == 3))
            nc.vector.tensor_copy(dst[:, 0, n, ch * 512:(ch + 1) * 512],
                                  ps[:])

    def v_proj(yb):
        for oc in range(2):
            ps = psP.tile([128, 512], F32, tag="pP", name=f"pv{u}_{yb}_{oc}")
            for cb in range(4):
                nc.tensor.matmul(
                    ps[:],
                    f_sb[cb][:, yb * 128:(yb + 1) * 128],
                    wT[cb][:, 2048 + oc * 512:2048 + (oc + 1) * 512],
                    start=(cb == 0), stop=(cb == 3))
            psv = ps.rearrange("p (n d) -> p n d", n=4)
            nc.vector.tensor_copy(v2v[:, yb, oc * 4:(oc + 1) * 4, 0:128],
                                  psv[:])

    E = {}

    def qk_head(n, j):
        ps = ps_l.tile([128, 1024], F32, tag="l", name=f"l{u}_{n}_{j}")
        for ch in range(2):
            nc.tensor.matmul(
                ps[:, ch * 512:(ch + 1) * 512],
                K8i[:, :, n, j * 128:(j + 1) * 128],
                Q8i[:, :, n, ch * 512:(ch + 1) * 512],
                start=True, stop=True, perf_mode=DR)
        e = ep.tile([128, 1024], BF16, tag="e", name=f"e{u}_{n}_{j}")
        nc.scalar.activation(e[:], ps[:], EXP, scale=SCALE)
        E[(n, j)] = e

    # ---- q projections, then G, then pipelined attention ---------------
    for ob in range(8):
        qk_proj(ob)

    # G gather matrices into Q8 subtile 1:
    # Gw[b, x] = Lw[x, b - w(x) + 31] -> rows 64-95; Gh -> rows 96-127.
    # Two sub-phases (all Gw, then all Gh): a Gh matmul's read of
    # Q8[:, 1, (n, hh, :)] overlaps every Gw eviction, so Gh waits for
    # the Gw sub-phase; within a sub-phase, column classes (w(x) == ww)
    # are disjoint. All evictions on DVE: Act must stay a pure exp
    # chain (it is the steady-state pacing engine).
    for ww in range(W):
        ps = psP.tile([128, 512], F32, tag="pP", name=f"gw{u}_{ww}")
        nc.tensor.matmul(ps[0:32, 0:256], rwv[:, :, 31 - ww:63 - ww],
                         Q8g[:, :, :, :, ww], start=True, stop=True,
                         perf_mode=DR)
        nc.vector.tensor_copy(Q8g[64:96, 1, :, :, ww], ps[0:32, 0:256])
    for hh in range(H):
        ps = psP.tile([128, 512], F32, tag="pP", name=f"gh{u}_{hh}")
        nc.tensor.matmul(ps[0:32, 0:256], rhv[:, :, 31 - hh:63 - hh],
                         Q8g[:, :, :, hh, :], start=True, stop=True,
                         perf_mode=DR)
        nc.vector.tensor_copy(Q8g[96:128, 1, :, hh, :], ps[0:32, 0:256])

    # first k head + its logits
    qk_proj(8)
    for j in range(8):
        qk_head(0, j)

    for n in range(NH):
        # lookahead: next head's k projection + first 4 logit tiles keeps
        # the Act exp chain fed while this head's AV runs
        if n + 1 < NH:
            qk_proj(8 + n + 1)
            for j in range(4):
                qk_head(n + 1, j)
        if n == 0:
            for yb in range(8):
                v_proj(yb)

        for g4 in range(2):
            # transpose staging: bf16 view of a psP slot
            ptt = psP.tile([128, 512], F32, tag="pP", name=f"pt{u}_{n}_{g4}")
            pt = ptt.bitcast(BF16)[:, 0:512]
            for pair in range(2):
                pso = psP.tile([128, 512], F32, tag="pP",
                               name=f"o{u}_{n}_{g4}_{pair}")
                for sub in range(2):
                    xb = g4 * 4 + pair * 2 + sub
                    reg = pso[:, sub * 129:(sub + 1) * 129]
                    for j in range(8):
                        nc.tensor.matmul(
                            reg,
                            E[(n, j)][:, xb * 128:(xb + 1) * 128],
                            v2v[:, j, n, :],
                            start=(j == 0), stop=(j == 7))
                for sub in range(2):
                    xb = g4 * 4 + pair * 2 + sub
                    c0 = sub * 129
                    rz = rzp.tile([128, 1], F32, tag="rz",
                                  name=f"rz{u}_{n}_{xb}")
                    nc.vector.reciprocal(rz[:], pso[:, c0 + 128:c0 + 129])
                    osb = osp.tile([128, 128], BF16, tag="osb",
                                   name=f"osb{u}_{n}_{xb}")
                    nc.vector.tensor_scalar_mul(osb[:], pso[:, c0:c0 + 128],
                                                rz[:])
                    nc.tensor.transpose(
                        pt[:, (pair * 2 + sub) * 128:
                           (pair * 2 + sub + 1) * 128],
                        osb[:], ident[:])
            ob = obp.tile([128, 512], BF16, tag="ob", name=f"ob{u}_{n}_{g4}")
            nc.vector.tensor_copy(ob[:], pt[:])
            # out-DMAs go via the idle gpsimd queue: SP's in-order DMA
            # queue must stay clear for next-iteration input DMAs
            nc.gpsimd.dma_start(
                out_d[n * 128:(n + 1) * 128, g4 * 512:(g4 + 1) * 512],
                ob[:])

        if n + 1 < NH:
            for j in range(4, 8):
                qk_head(n + 1, j)


def _consts():
    import ml_dtypes
    F8NP = ml_dtypes.float8_e4m3
    BFNP = ml_dtypes.bfloat16

    def rel_split(rel):
        # [128, 2, 64] fp8: subtile 0 = rel^T (full d=128), subtile 1 = 0
        pad = np.zeros((64, 128), np.float32)
        pad[:63] = rel
        t = np.zeros((128, 2, 64), np.float32)
        t[:, 0, :] = pad.T            # [d, j]
        return np.ascontiguousarray(t.reshape(128, 128)).astype(F8NP)

    x = np.arange(HW)
    hh, ww = np.divmod(x, W)
    oh = np.zeros((128, NH, HW), np.float32)
    for b in range(32):
        oh[64 + b, :, :] = (ww == b)[None, :]
        oh[96 + b, :, :] = (hh == b)[None, :]
    oh = oh.reshape(128, NH * HW).astype(F8NP)

    ident = np.eye(128, dtype=np.float32).astype(BFNP)
    return rel_split, oh, ident


def make_in_maps(featuremap, w_qkv, rel_height, rel_width):
    import ml_dtypes
    BFNP = ml_dtypes.bfloat16

    rel_split, oh, ident = _consts()
    wT = np.ascontiguousarray(np.asarray(w_qkv, np.float32).T).astype(BFNP)
    relw = rel_split(np.asarray(rel_width, np.float32))
    relh = rel_split(np.asarray(rel_height, np.float32))
    fm = np.asarray(featuremap, np.float32)
    maps = []
    for b in range(B):
        maps.append({
            "f": np.ascontiguousarray(fm[b].reshape(C, HW)).astype(BFNP),
            "wT": wT, "relw": relw, "relh": relh,
            "oh": oh, "ident": ident,
        })
    return maps


_NC_CACHE = {}


def get_nc():
    if "nc" not in _NC_CACHE:
        _NC_CACHE["nc"] = build_nc()
    return _NC_CACHE["nc"]


def kernel(featuremap, w_qkv, rel_height, rel_width):
    from concourse.bass_utils import run_bass_kernel_spmd

    nc = get_nc()
    in_maps = make_in_maps(featuremap, w_qkv, rel_height, rel_width)
    res = run_bass_kernel_spmd(nc, in_maps, list(range(B)))
    out = np.stack([np.asarray(res.results[b]["out"], dtype=np.float32)
                    for b in range(B)])
    return out.reshape(B, NH * D, H, W)


if __name__ == "__main__":
    nc = build_nc()
    print("built ok:", len(nc.m.functions[0].blocks), "blocks")
